# revision 32
# baseline (speedup 1.0000x reference)
"""Fused pre-LN multi-head self-attention block for Trainium2, SPMD over 8 NeuronCores.

Strategy (Megatron-style head parallelism):
  - Each core owns 2 of the 16 heads (a 128-wide slice of the QKV output dims)
    for BOTH batches, and computes a partial dense-projection output; the host
    sums the 8 partials and adds bd.
  - hidden_states is shipped transposed (xT [1024, 4096], bf16) so the
    normalization is algebraically folded into the QKV projections:
        q = rstd * (x @ WqeT - mu * wqsum) + bqe
    with Wqe = Wq_slice * gamma * scale (host-folded), bqe = Wq_slice @ beta + bq.
  - LayerNorm statistics are mostly OFF the PE: ScalarE squares x; the DVE
    tree-folds the 8 hid-chunks of x to one [128,512] tile; the PE finishes
    with 1 ones-matmul for sum(x) and 8 for sum(x^2) (9 vs the naive 16).
    The row chain (var, sqrt, reciprocal) runs single-lane per pg from the
    PSUM stats row (evacuated by ScalarE); rstd is broadcast by GpSimd and
    transposed to per-position columns with tiny PE transposes for V scaling.
  - The rstd scaling of V happens during the V-transpose evacuation copies
    (per-partition tensor_scalar); vT's PSUM->SBUF evacuation runs on the
    otherwise idle ScalarE.  (GpSimd bulk reductions and fp8 DoubleRow scores
    were both tried and measured SLOWER / too inaccurate on HW.)
  - Scores are computed transposed (sT[k,q] = kT.T @ qT); softmax skips the
    max-subtraction (inputs are standard-normal scale); the denominator comes
    from a ones-column inside the padded V tiles in the probs@V matmul.
  - Both reciprocals (1/std, 1/denominator) use the single-op DVE
    reciprocal_approx_fast.
  - PSUM pools are scoped per phase; phase 2 keeps scores double-buffered
    (2x2 banks) + ctx 2 + dense 2.
  - V tiles are zero-padded to 128 lhsT columns (ones col at 64) so the
    probs@V weight loads qualify for fast-weight-load and overlap matmuls.
"""

import sys

sys.path.insert(0, "/opt/trn_rl_repo")

import numpy as np
import ml_dtypes

B, S, HID = 2, 2048, 1024
NH, HD = 16, 64
EPS = 1e-12
NCORES = 8
DL = HID // NCORES          # 128 local q/k/v dims (2 heads) per core
PB = B * S                  # 4096 total positions
SCALE = 1.0 / np.sqrt(HD)   # combined q*k score scale (1/8), folded into Wq
NPOSG = PB // 512           # 8 position groups of 512
KC = S // 128               # 16 key chunks per batch
QG = S // 512               # 4 query groups per batch

_BUILT = {}
last_launch = {}


def _build(with_bias, fused_mask):
    import concourse.tile as tile
    from concourse import bacc, bass_isa, mybir
    from contextlib import ExitStack

    F32 = mybir.dt.float32
    BF16 = mybir.dt.bfloat16
    F16 = mybir.dt.float16
    AF = mybir.ActivationFunctionType
    OP = mybir.AluOpType
    AX = mybir.AxisListType

    nc = bacc.Bacc("TRN2", target_bir_lowering=False, debug=False)

    xT = nc.dram_tensor("xT", [HID, PB], BF16, kind="ExternalInput")
    wq = nc.dram_tensor("wq", [HID, DL], BF16, kind="ExternalInput")
    wk = nc.dram_tensor("wk", [HID, DL], BF16, kind="ExternalInput")
    wv = nc.dram_tensor("wv", [HID, DL], BF16, kind="ExternalInput")
    wd = nc.dram_tensor("wd", [DL, HID], BF16, kind="ExternalInput")
    # raw bf16-weight column sums as three [1, DL] rows (q, k, v) for the
    # rank-1 mean-correction matmul
    wsumsT = nc.dram_tensor("wsumsT", [1, 3 * DL], BF16, kind="ExternalInput")
    if not fused_mask:
        madd = nc.dram_tensor("madd", [128, B * KC], F32, kind="ExternalInput")
    if with_bias:
        bqkv = nc.dram_tensor("bqkv", [DL, 3], F32, kind="ExternalInput")
    out = nc.dram_tensor("out", [PB, HID], F16, kind="ExternalOutput")

    with tile.TileContext(nc) as tc, ExitStack() as ctx:
        consts = ctx.enter_context(tc.tile_pool(name="consts", bufs=1))
        persist = ctx.enter_context(tc.tile_pool(name="persist", bufs=1))
        xpool = ctx.enter_context(tc.tile_pool(name="xpool", bufs=3))
        sqp = ctx.enter_context(tc.tile_pool(name="sqp", bufs=2))
        trp = ctx.enter_context(tc.tile_pool(name="trp", bufs=2))
        rowp = ctx.enter_context(tc.tile_pool(name="rowp", bufs=4))
        murw = ctx.enter_context(tc.tile_pool(name="murw", bufs=4))
        bcp = ctx.enter_context(tc.tile_pool(name="bcp", bufs=2))
        epp = ctx.enter_context(tc.tile_pool(name="epp", bufs=3))
        etp = ctx.enter_context(tc.tile_pool(name="etp", bufs=6))
        dnp = ctx.enter_context(tc.tile_pool(name="dnp", bufs=2))
        obp = ctx.enter_context(tc.tile_pool(name="obp", bufs=4))

        # ---- constants / weights
        ident32 = consts.tile([128, 128], F32)
        from concourse.masks import make_identity
        make_identity(nc, ident32[:])
        ident = consts.tile([128, 128], BF16)
        nc.vector.tensor_copy(ident[:], ident32[:])
        ones_col = consts.tile([128, 1], BF16)
        nc.vector.memset(ones_col[:], 1.0)
        eps_t = consts.tile([1, 1], F32)
        nc.vector.memset(eps_t[:], EPS)
        wsumsT_sb = consts.tile([1, 3 * DL], BF16)
        nc.sync.dma_start(out=wsumsT_sb[:], in_=wsumsT[:, :])
        if not fused_mask:
            madd_sb = consts.tile([128, B * KC], F32)
            nc.sync.dma_start(out=madd_sb[:], in_=madd[:, :])
        if with_bias:
            bqkv_sb = consts.tile([DL, 3], F32)
            nc.sync.dma_start(out=bqkv_sb[:], in_=bqkv[:, :])

        xT_r = xT.rearrange("(hc p) q -> p hc q", p=128)
        x_tiles = {}
        for pg in range(2):
            x_tiles[pg] = xpool.tile([128, 8, 512], BF16, tag="x", name=f"x{pg}")
            nc.sync.dma_start(
                out=x_tiles[pg][:], in_=xT_r[:, :, pg * 512 : pg * 512 + 512]
            )

        wq_sb = persist.tile([128, 8, DL], BF16)
        wk_sb = persist.tile([128, 8, DL], BF16)
        wv_sb = persist.tile([128, 8, DL], BF16)
        for wsb, wdr in ((wq_sb, wq), (wk_sb, wk), (wv_sb, wv)):
            nc.sync.dma_start(
                out=wsb[:], in_=wdr.rearrange("(hc p) d -> p hc d", p=128)
            )
        wd_sb = persist.tile([DL, HID], BF16)
        nc.sync.dma_start(out=wd_sb[:], in_=wd[:, :])

        qT_sb = persist.tile([128, PB], BF16)
        kT_sb = persist.tile([128, PB], BF16)
        # v tiles padded to 128 lhsT columns: [0:64]=v, [64]=ones, [65:128]=0
        vA_sb = persist.tile([128, B * KC, 128], BF16)
        vB_sb = persist.tile([128, B * KC, 128], BF16)
        for vsb in (vA_sb, vB_sb):
            # on GpSimd: these pads sit ahead of the first stats tree in the
            # DVE program order and would delay phase-1 startup by ~4us
            nc.gpsimd.memset(vsb[:, :, HD : HD + 1], 1.0)
            nc.gpsimd.memset(vsb[:, :, HD + 1 : 128], 0.0)
        ctxT_sb = persist.tile([128, PB], BF16)


        # ================= phase 1: LN stats + QKV projections ===============
        # stats run on Scalar (x^2), DVE (x-chunk tree), GpSimd (x^2 tree +
        # partition reductions); the PE only does QKV matmuls + transposes.
        # Row chains (var/sqrt/recip) are batched per 2 pgs (multi-lane).
        with tc.tile_pool(name="ph1ps", bufs=1, space="PSUM") as ph1:

            stats_ps_map = {}

            def emit_stats(pg):
                # s1 via a DVE add-tree + ONE ones-matmul; s2 via 8 ones-
                # matmuls over x^2 chunks (x^2 on the idle ScalarE).  Both
                # land in one [1,1024] PSUM row tile (s1 cols 0:512, s2
                # 512:1024).  GpSimd proved too slow for bulk reduction work.
                xa = x_tiles[pg]
                xsq = sqp.tile([128, 8, 512], BF16, tag="xsq", name=f"xsq{pg}")
                nc.scalar.activation(xsq[:], xa[:], AF.Square)
                y1 = trp.tile([128, 4, 512], BF16, tag="y1", name=f"y1_{pg}")
                nc.vector.tensor_add(y1[:], xa[:, 0:4, :], xa[:, 4:8, :])
                y2 = trp.tile([128, 2, 512], BF16, tag="y2", name=f"y2_{pg}")
                nc.vector.tensor_add(y2[:], y1[:, 0:2, :], y1[:, 2:4, :])
                yy = trp.tile([128, 512], BF16, tag="yy", name=f"yy{pg}")
                nc.vector.tensor_add(yy[:], y2[:, 0, :], y2[:, 1, :])
                sp = ph1.tile([1, 1024], F32, tag="stats", bufs=1, name=f"sp{pg}")
                nc.tensor.matmul(
                    sp[0:1, 0:512], lhsT=ones_col[:], rhs=yy[:],
                    start=True, stop=True, skip_group_check=True,
                )
                for hc in range(8):
                    nc.tensor.matmul(
                        sp[0:1, 512:1024], lhsT=ones_col[:], rhs=xsq[:, hc, :],
                        start=(hc == 0), stop=(hc == 7), skip_group_check=True,
                    )
                stats_ps_map[pg] = sp

            mu_rows = {}
            rstd_rows = {}
            rstdT_sb = {}

            def emit_rows(pg):
                # single-lane row chain from the PSUM stats row; sqrt and mu
                # on ScalarE, the rest on DVE; rstd broadcast on GpSimd
                sp = stats_ps_map[pg]
                s_sb = rowp.tile([1, 1024], F32, tag="ssb", name=f"ssb{pg}")
                nc.scalar.copy(s_sb[:], sp[:])
                s1 = s_sb[0:1, 0:512]
                s2 = s_sb[0:1, 512:1024]
                t_sq = rowp.tile([1, 512], F32, tag="tsq", name=f"tsq{pg}")
                nc.vector.tensor_mul(t_sq[:], s1, s1)
                u_t = rowp.tile([1, 512], F32, tag="u", name=f"u{pg}")
                nc.vector.scalar_tensor_tensor(
                    out=u_t[:], in0=t_sq[:], scalar=1.0 / HID,
                    in1=s2, op0=OP.mult, op1=OP.subtract,
                )  # u = s1^2/HID - s2  (= -HID*var)
                std_t = rowp.tile([1, 512], F32, tag="std", name=f"std{pg}")
                nc.scalar.activation(
                    std_t[:], u_t[:], AF.Sqrt, bias=eps_t[:], scale=-1.0 / HID
                )
                rstd_row = rowp.tile([1, 512], F32, tag="rstd", name=f"rstd{pg}")
                nc.vector.reciprocal_approx_fast(out=rstd_row[:], in_=std_t[:])
                rstd_b = bcp.tile([128, 512], F32, tag="rstdb", name=f"rstdb{pg}")
                nc.gpsimd.partition_broadcast(rstd_b[:], rstd_row[:])
                rstd_rows[pg] = (rstd_row, rstd_b)
                mu_t = rowp.tile([1, 512], BF16, tag="mu", name=f"mu{pg}")
                nc.scalar.mul(mu_t[:], s1, -1.0 / HID)
                mu_rows[pg] = mu_t
                # rstd columns for the V scaling: 4 tiny transposes into one
                # PSUM tile, evacuated with a single [128,4] copy
                tp = ph1.tile([128, 4], F32, tag="rTp", bufs=1, name=f"rTp{pg}")
                for c in range(4):
                    nc.tensor.matmul(
                        tp[:, c : c + 1], rstd_row[:, c * 128 : (c + 1) * 128],
                        ident32[0:1, 0:1], is_transpose=True,
                        skip_group_check=True,
                    )
                rT = bcp.tile([128, 4], F32, tag="rT", name=f"rT{pg}")
                nc.vector.tensor_copy(rT[:], tp[:])
                rstdT_sb[pg] = rT

            qkv_open = {}

            def emit_qkv_mms(pg):
                # the 24 weight matmuls depend only on x + weights — emitting
                # them before the stats folds lets the PE start ~8us earlier
                # for pg0/pg1 (three PSUM groups stay open until the rank-1
                # corrections in emit_qkv close them)
                xa = x_tiles[pg]
                tiles = []
                for w_sb in (wq_sb, wk_sb, wv_sb):
                    mm_ps = ph1.tile([128, 512], F32, tag="mm", bufs=3)
                    for hc in range(8):
                        nc.tensor.matmul(
                            mm_ps[:], lhsT=w_sb[:, hc, :], rhs=xa[:, hc, :],
                            start=(hc == 0), stop=False,
                            skip_group_check=True,
                        )
                    tiles.append(mm_ps)
                qkv_open[pg] = tiles

            def emit_qkv(pg):
                rstd_b = rstd_rows[pg][1]
                ps = pg * 512
                if pg not in qkv_open:
                    emit_qkv_mms(pg)
                tiles = qkv_open.pop(pg)
                vT_blk = epp.tile([128, 512], BF16, tag="vT", name=f"vT{pg}")
                # with bias, V must be rstd-scaled before the bias add, so it
                # takes the DVE stt path; otherwise V is evacuated raw on the
                # idle ScalarE and rstd is applied after the transpose.
                for (mm_ps, wcol, target, scaled) in (
                    (tiles[0], 0, qT_sb[:, ps : ps + 512], True),
                    (tiles[1], 1, kT_sb[:, ps : ps + 512], True),
                    (tiles[2], 2, vT_blk[:], bool(with_bias)),
                ):
                    nc.tensor.matmul(
                        mm_ps[:],
                        lhsT=wsumsT_sb[:, wcol * DL : (wcol + 1) * DL],
                        rhs=mu_rows[pg][:], start=False, stop=True,
                        skip_group_check=True,
                    )
                    if scaled:
                        # target = raw_c * rstd (per-position column scale)
                        if with_bias:
                            t2 = epp.tile([128, 512], F32, tag="ep2")
                            nc.vector.scalar_tensor_tensor(
                                out=t2[:], in0=rstd_b[:], scalar=1.0,
                                in1=mm_ps[:], op0=OP.mult, op1=OP.mult,
                            )
                            nc.vector.tensor_scalar_add(
                                out=target, in0=t2[:],
                                scalar1=bqkv_sb[:, wcol : wcol + 1],
                            )
                        else:
                            nc.vector.scalar_tensor_tensor(
                                out=target, in0=rstd_b[:], scalar=1.0,
                                in1=mm_ps[:], op0=OP.mult, op1=OP.mult,
                            )
                    else:
                        # raw vT evacuation on the idle ScalarE; rstd is
                        # applied per-partition after the transpose
                        nc.scalar.copy(target, mm_ps[:])
                # transpose vT -> v (per 128-pos chunk) into padded v tiles,
                # scaling by rstd (now per-partition) on the way out
                for c4 in range(4):
                    j = pg * 4 + c4  # global 128-chunk == b*KC + kc
                    tp_ps = ph1.tile([128, 128], BF16, tag="vt", bufs=2)
                    nc.tensor.transpose(
                        tp_ps[:, :], vT_blk[:, c4 * 128 : (c4 + 1) * 128], ident[:]
                    )
                    if with_bias:
                        nc.vector.tensor_copy(vA_sb[:, j, 0:HD], tp_ps[:, 0:HD])
                        nc.vector.tensor_copy(
                            vB_sb[:, j, 0:HD], tp_ps[:, HD : 2 * HD]
                        )
                    else:
                        rcol = rstdT_sb[pg][:, c4 : c4 + 1]
                        nc.vector.tensor_scalar_mul(
                            out=vA_sb[:, j, 0:HD], in0=tp_ps[:, 0:HD], scalar1=rcol
                        )
                        nc.vector.tensor_scalar_mul(
                            out=vB_sb[:, j, 0:HD], in0=tp_ps[:, HD : 2 * HD],
                            scalar1=rcol,
                        )

            # software pipeline: stats/rows for pg+2/pg+1 emitted before
            # qkv(pg) so DVE/GpSimd/Scalar run ahead of the PE
            # warm the PE clock while the first x tiles stream in
            warm = ph1.tile([1, 1024], F32, tag="stats", bufs=1, name="warm")
            for _ in range(16):
                nc.tensor.matmul(
                    warm[0:1, 0:128], lhsT=ident32[:, 0:1], rhs=ident32[:],
                    start=True, stop=True, skip_group_check=True,
                )
            emit_qkv_mms(0)
            emit_stats(0)
            emit_rows(0)
            emit_stats(1)
            emit_rows(1)
            for pg in range(NPOSG):
                nxt = pg + 2
                if nxt < NPOSG:
                    x_tiles[nxt] = xpool.tile(
                        [128, 8, 512], BF16, tag="x", name=f"x{nxt}"
                    )
                    nc.sync.dma_start(
                        out=x_tiles[nxt][:],
                        in_=xT_r[:, :, nxt * 512 : nxt * 512 + 512],
                    )
                    emit_stats(nxt)
                    emit_rows(nxt)
                emit_qkv(pg)

        # ================= phase 2: attention + pipelined dense ==============
        with tc.tile_pool(name="scps", bufs=1, space="PSUM") as scps, \
             tc.tile_pool(name="ctps", bufs=1, space="PSUM") as ctps, \
             tc.tile_pool(name="dsps", bufs=1, space="PSUM") as dsps:

            def emit_dense_piece(qs, piece):
                # one (pos-chunk, half) slice of the dense projection; pieces
                # are woven one-per-unit into the NEXT query group so the
                # 8-matmul dense block never head-of-line-blocks the next
                # group's scores in the in-order PE queue
                c4, half = piece // 2, piece % 2
                pc = qs + c4 * 128
                ops_ = dsps.tile([128, 512], F32, tag="ds", bufs=2)
                nc.tensor.matmul(
                    ops_[:], lhsT=ctxT_sb[:, pc : pc + 128],
                    rhs=wd_sb[:, half * 512 : (half + 1) * 512],
                    start=True, stop=True,
                )
                osb = obp.tile([128, 512], F16, tag="ob")
                nc.vector.tensor_copy(osb[:], ops_[:])
                nc.sync.dma_start(
                    out=out[pc : pc + 128, half * 512 : (half + 1) * 512],
                    in_=osb[:],
                )

            pending_qs = None
            for b in range(B):
                for qg in range(QG):
                    qs = b * S + qg * 512
                    ctxA_ps = ctps.tile([128, 512], F32, tag="ctx", bufs=2)
                    ctxB_ps = ctps.tile([128, 512], F32, tag="ctx", bufs=2)
                    # per-head sub-units (scores -> exp -> pv) so the two sc
                    # slots recycle alternately and ScalarE never bubbles
                    for kc2 in range(KC // 2):
                        kc = 2 * kc2
                        ks = b * S + kc * 128
                        st = kc == 0
                        sp2 = kc + 1 == KC - 1
                        for h, (kh, vh, cps) in enumerate(
                            ((slice(0, 64), vA_sb, ctxA_ps),
                             (slice(64, 128), vB_sb, ctxB_ps))
                        ):
                            u = 2 * kc2 + h
                            if pending_qs is not None and 6 <= u < 14:
                                emit_dense_piece(pending_qs, u - 6)
                            elif pending_qs is None and u < 12:
                                # first query group has no woven dense: keep
                                # the PE stream dense with throwaway matmuls
                                # so it ramps to full clock instead of
                                # settling into a slow-PE/late-scores loop
                                dmy = dsps.tile(
                                    [128, 512], F32, tag="ds", bufs=2,
                                    name=f"dmy{u}",
                                )
                                nc.tensor.matmul(
                                    dmy[:], lhsT=ident[:],
                                    rhs=wd_sb[:, 0:512],
                                    start=True, stop=True,
                                )
                            psH = scps.tile(
                                [128, 1024], F32, tag="sc", bufs=2, name=f"ps{h}"
                            )
                            for j in range(2):
                                nc.tensor.matmul(
                                    psH[:, 512 * j : 512 * (j + 1)],
                                    lhsT=kT_sb[kh, ks + 128 * j : ks + 128 * (j + 1)],
                                    rhs=qT_sb[kh, qs : qs + 512],
                                    start=True, stop=True,
                                )
                            eH = etp.tile([128, 1024], BF16, tag="e", name=f"e{h}")
                            if fused_mask:
                                nc.scalar.activation(eH[:], psH[:], AF.Exp)
                            else:
                                for j in range(2):
                                    mcol = madd_sb[
                                        :, b * KC + kc + j : b * KC + kc + j + 1
                                    ]
                                    nc.scalar.activation(
                                        eH[:, 512 * j : 512 * (j + 1)],
                                        psH[:, 512 * j : 512 * (j + 1)],
                                        AF.Exp, bias=mcol, scale=1.0,
                                    )
                            for j in range(2):
                                nc.tensor.matmul(
                                    cps[:, :],
                                    lhsT=vh[:, b * KC + kc + j, :],
                                    rhs=eH[:, 512 * j : 512 * (j + 1)],
                                    start=(st and j == 0), stop=(sp2 and j == 1),
                                )

                    # evacuate ctx PSUM fast (frees the accum slots for the
                    # next query group), then normalize from the SBUF copies:
                    # head A ctx -> partitions 0:64, head B ctx -> 64:128 so the
                    # normalize muls have partition-aligned SBUF operands
                    cAB = dnp.tile([128, 512], F32, tag="cs", bufs=2)
                    nc.vector.tensor_copy(cAB[0:HD, :], ctxA_ps[0:HD, :])
                    nc.vector.tensor_copy(cAB[HD : 2 * HD, :], ctxB_ps[0:HD, :])
                    dn_row = dnp.tile([1, 1024], F32, tag="dn_row", bufs=2)
                    nc.vector.tensor_copy(dn_row[:, 0:512], ctxA_ps[HD : HD + 1, :])
                    nc.vector.tensor_copy(dn_row[:, 512:1024], ctxB_ps[HD : HD + 1, :])
                    rdn_row = dnp.tile([1, 1024], F32, tag="rdn_row", bufs=1)
                    nc.vector.reciprocal_approx_fast(out=rdn_row[:], in_=dn_row[:])
                    rdn_b = dnp.tile([128, 1024], F32, tag="rdn_b", bufs=1)
                    nc.gpsimd.partition_broadcast(rdn_b[:], rdn_row[:])
                    nc.vector.tensor_mul(
                        ctxT_sb[0:HD, qs : qs + 512],
                        cAB[0:HD, :], rdn_b[0:HD, 0:512],
                    )
                    nc.vector.tensor_mul(
                        ctxT_sb[HD : 2 * HD, qs : qs + 512],
                        cAB[HD : 2 * HD, :], rdn_b[HD : 2 * HD, 512:1024],
                    )
                    pending_qs = qs
            # dense for the final query group has no successor to hide in
            for piece in range(8):
                emit_dense_piece(pending_qs, piece)
    nc.compile()
    return nc


def _get_nc(with_bias, fused_mask):
    key = (bool(with_bias), bool(fused_mask))
    if key not in _BUILT:
        _BUILT[key] = _build(*key)
    return _BUILT[key]


def kernel(
    hidden_states,
    attention_mask,
    Wq, bq, Wk, bk, Wv, bv, Wd, bd,
    ln_gamma, ln_beta,
):
    from concourse.bass_utils import run_bass_kernel_spmd

    hidden_states = np.asarray(hidden_states, dtype=np.float32)
    attention_mask = np.asarray(attention_mask, dtype=np.float32)
    Wq, bq = np.asarray(Wq, np.float32), np.asarray(bq, np.float32)
    Wk, bk = np.asarray(Wk, np.float32), np.asarray(bk, np.float32)
    Wv, bv = np.asarray(Wv, np.float32), np.asarray(bv, np.float32)
    Wd, bd = np.asarray(Wd, np.float32), np.asarray(bd, np.float32)
    gamma = np.asarray(ln_gamma, np.float32)
    beta = np.asarray(ln_beta, np.float32)

    x2d = hidden_states.reshape(PB, HID)
    xT = np.ascontiguousarray(x2d.T).astype(ml_dtypes.bfloat16)

    ma = (-1000.0 * (1.0 - attention_mask)).astype(np.float32)  # [B, S]
    madd = np.ascontiguousarray(
        ma.reshape(B, KC, 128).transpose(2, 0, 1).reshape(128, B * KC)
    )
    fused_mask = not np.any(ma != 0)

    in_maps = []
    biases_eff = []
    for p in range(NCORES):
        sl = slice(DL * p, DL * (p + 1))
        wq_e = Wq[sl, :] * gamma[None, :] * np.float32(SCALE)
        wk_e = Wk[sl, :] * gamma[None, :]
        wv_e = Wv[sl, :] * gamma[None, :]
        wq_b = np.ascontiguousarray(wq_e.T).astype(ml_dtypes.bfloat16)
        wk_b = np.ascontiguousarray(wk_e.T).astype(ml_dtypes.bfloat16)
        wv_b = np.ascontiguousarray(wv_e.T).astype(ml_dtypes.bfloat16)
        # raw column sums of the bf16 weights actually used on device,
        # as three [1, DL] rows for the rank-1 mean-correction matmul
        wsumsT = np.concatenate(
            [
                wq_b.astype(np.float32).sum(axis=0),
                wk_b.astype(np.float32).sum(axis=0),
                wv_b.astype(np.float32).sum(axis=0),
            ]
        ).reshape(1, 3 * DL).astype(ml_dtypes.bfloat16)
        b_eff = np.stack(
            [
                (Wq[sl, :] @ beta + bq[sl]) * np.float32(SCALE),
                Wk[sl, :] @ beta + bk[sl],
                Wv[sl, :] @ beta + bv[sl],
            ],
            axis=1,
        ).astype(np.float32)
        biases_eff.append(b_eff)
        wd_s = np.ascontiguousarray(Wd[:, sl].T).astype(ml_dtypes.bfloat16)
        in_maps.append(
            {
                "xT": xT,
                "wq": wq_b,
                "wk": wk_b,
                "wv": wv_b,
                "wd": wd_s,
                "wsumsT": wsumsT,
            }
        )

    with_bias = any(np.any(b != 0) for b in biases_eff)
    if with_bias:
        for p in range(NCORES):
            in_maps[p]["bqkv"] = biases_eff[p]
    if not fused_mask:
        for p in range(NCORES):
            in_maps[p]["madd"] = madd

    nc = _get_nc(with_bias, fused_mask)
    last_launch["nc"] = nc
    last_launch["in_maps"] = in_maps
    res = run_bass_kernel_spmd(nc, in_maps, core_ids=list(range(NCORES)))
    acc = res.results[0]["out"].astype(np.float32).copy()
    for p in range(1, NCORES):
        acc += res.results[p]["out"]
    acc += bd[None, :]
    return acc.reshape(B, S, HID)


# revision 33
# speedup vs baseline: 1.0376x; 1.0376x over previous
"""Fused pre-LN multi-head self-attention block for Trainium2, SPMD over 8 NeuronCores.

Strategy (Megatron-style head parallelism):
  - Each core owns 2 of the 16 heads (a 128-wide slice of the QKV output dims)
    for BOTH batches, and computes a partial dense-projection output; the host
    sums the 8 partials and adds bd.
  - hidden_states is shipped transposed (xT [1024, 4096], bf16) so the
    normalization is algebraically folded into the QKV projections:
        q = rstd * (x @ WqeT - mu * wqsum) + bqe
    with Wqe = Wq_slice * gamma * scale (host-folded), bqe = Wq_slice @ beta + bq.
  - LayerNorm statistics are mostly OFF the PE: ScalarE squares x; the DVE
    tree-folds the 8 hid-chunks of x to one [128,512] tile; the PE finishes
    with 1 ones-matmul for sum(x) and 8 for sum(x^2) (9 vs the naive 16).
    The row chain (var, sqrt, reciprocal) runs single-lane per pg from the
    PSUM stats row (evacuated by ScalarE); rstd is broadcast by GpSimd and
    transposed to per-position columns with tiny PE transposes for V scaling.
  - The rstd scaling of V happens during the V-transpose evacuation copies
    (per-partition tensor_scalar); vT's PSUM->SBUF evacuation runs on the
    otherwise idle ScalarE.  (GpSimd bulk reductions and fp8 DoubleRow scores
    were both tried and measured SLOWER / too inaccurate on HW.)
  - Scores are computed transposed (sT[k,q] = kT.T @ qT); softmax skips the
    max-subtraction (inputs are standard-normal scale); the denominator comes
    from a ones-column inside the padded V tiles in the probs@V matmul.
  - Both reciprocals (1/std, 1/denominator) use the single-op DVE
    reciprocal_approx_fast.
  - PSUM pools are scoped per phase; phase 2 keeps scores double-buffered
    (2x2 banks) + ctx 2 + dense 2.
  - V tiles are zero-padded to 128 lhsT columns (ones col at 64) so the
    probs@V weight loads qualify for fast-weight-load and overlap matmuls.
"""

import sys

sys.path.insert(0, "/opt/trn_rl_repo")

import numpy as np
import ml_dtypes

B, S, HID = 2, 2048, 1024
NH, HD = 16, 64
EPS = 1e-12
NCORES = 8
DL = HID // NCORES          # 128 local q/k/v dims (2 heads) per core
PB = B * S                  # 4096 total positions
SCALE = 1.0 / np.sqrt(HD)   # combined q*k score scale (1/8), folded into Wq
NPOSG = PB // 512           # 8 position groups of 512
KC = S // 128               # 16 key chunks per batch
QG = S // 512               # 4 query groups per batch

_BUILT = {}
last_launch = {}


def _build(with_bias, fused_mask):
    import concourse.tile as tile
    from concourse import bacc, bass_isa, mybir
    from contextlib import ExitStack

    F32 = mybir.dt.float32
    BF16 = mybir.dt.bfloat16
    F16 = mybir.dt.float16
    AF = mybir.ActivationFunctionType
    OP = mybir.AluOpType
    AX = mybir.AxisListType

    nc = bacc.Bacc("TRN2", target_bir_lowering=False, debug=False)

    xT = nc.dram_tensor("xT", [HID, PB], BF16, kind="ExternalInput")
    wq = nc.dram_tensor("wq", [HID, DL], BF16, kind="ExternalInput")
    wk = nc.dram_tensor("wk", [HID, DL], BF16, kind="ExternalInput")
    wv = nc.dram_tensor("wv", [HID, DL], BF16, kind="ExternalInput")
    wd = nc.dram_tensor("wd", [DL, HID], BF16, kind="ExternalInput")
    # raw bf16-weight column sums as three [1, DL] rows (q, k, v) for the
    # rank-1 mean-correction matmul
    wsumsT = nc.dram_tensor("wsumsT", [1, 3 * DL], BF16, kind="ExternalInput")
    if not fused_mask:
        madd = nc.dram_tensor("madd", [128, B * KC], F32, kind="ExternalInput")
    if with_bias:
        bqkv = nc.dram_tensor("bqkv", [DL, 3], F32, kind="ExternalInput")
    out = nc.dram_tensor("out", [PB, HID], F16, kind="ExternalOutput")

    with tile.TileContext(nc) as tc, ExitStack() as ctx:
        consts = ctx.enter_context(tc.tile_pool(name="consts", bufs=1))
        persist = ctx.enter_context(tc.tile_pool(name="persist", bufs=1))
        xpool = ctx.enter_context(tc.tile_pool(name="xpool", bufs=3))
        sqp = ctx.enter_context(tc.tile_pool(name="sqp", bufs=2))
        trp = ctx.enter_context(tc.tile_pool(name="trp", bufs=2))
        rowp = ctx.enter_context(tc.tile_pool(name="rowp", bufs=4))
        murw = ctx.enter_context(tc.tile_pool(name="murw", bufs=4))
        bcp = ctx.enter_context(tc.tile_pool(name="bcp", bufs=2))
        epp = ctx.enter_context(tc.tile_pool(name="epp", bufs=3))
        etp = ctx.enter_context(tc.tile_pool(name="etp", bufs=6))
        dnp = ctx.enter_context(tc.tile_pool(name="dnp", bufs=2))
        obp = ctx.enter_context(tc.tile_pool(name="obp", bufs=4))

        # ---- constants / weights
        ident32 = consts.tile([128, 128], F32)
        from concourse.masks import make_identity
        make_identity(nc, ident32[:])
        ident = consts.tile([128, 128], BF16)
        nc.vector.tensor_copy(ident[:], ident32[:])
        ones_col = consts.tile([128, 1], BF16)
        nc.vector.memset(ones_col[:], 1.0)
        eps_t = consts.tile([1, 1], F32)
        nc.vector.memset(eps_t[:], EPS)
        wsumsT_sb = consts.tile([1, 3 * DL], BF16)
        nc.sync.dma_start(out=wsumsT_sb[:], in_=wsumsT[:, :])
        if not fused_mask:
            madd_sb = consts.tile([128, B * KC], F32)
            nc.sync.dma_start(out=madd_sb[:], in_=madd[:, :])
        if with_bias:
            bqkv_sb = consts.tile([DL, 3], F32)
            nc.sync.dma_start(out=bqkv_sb[:], in_=bqkv[:, :])

        xT_r = xT.rearrange("(hc p) q -> p hc q", p=128)
        x_tiles = {}
        for pg in range(2):
            x_tiles[pg] = xpool.tile([128, 8, 512], BF16, tag="x", name=f"x{pg}")
            nc.sync.dma_start(
                out=x_tiles[pg][:], in_=xT_r[:, :, pg * 512 : pg * 512 + 512]
            )

        wq_sb = persist.tile([128, 8, DL], BF16)
        wk_sb = persist.tile([128, 8, DL], BF16)
        wv_sb = persist.tile([128, 8, DL], BF16)
        for wsb, wdr in ((wq_sb, wq), (wk_sb, wk), (wv_sb, wv)):
            nc.sync.dma_start(
                out=wsb[:], in_=wdr.rearrange("(hc p) d -> p hc d", p=128)
            )
        wd_sb = persist.tile([DL, HID], BF16)
        nc.sync.dma_start(out=wd_sb[:], in_=wd[:, :])

        qT_sb = persist.tile([128, PB], BF16)
        kT_sb = persist.tile([128, PB], BF16)
        # v tiles padded to 128 lhsT columns: [0:64]=v, [64]=ones, [65:128]=0
        vA_sb = persist.tile([128, B * KC, 128], BF16)
        vB_sb = persist.tile([128, B * KC, 128], BF16)
        for vsb in (vA_sb, vB_sb):
            # on GpSimd: these pads sit ahead of the first stats tree in the
            # DVE program order and would delay phase-1 startup by ~4us
            nc.gpsimd.memset(vsb[:, :, HD : HD + 1], 1.0)
            nc.gpsimd.memset(vsb[:, :, HD + 1 : 128], 0.0)
        ctxT_sb = persist.tile([128, PB], BF16)


        # ================= phase 1: LN stats + QKV projections ===============
        # stats run on Scalar (x^2), DVE (x-chunk tree), GpSimd (x^2 tree +
        # partition reductions); the PE only does QKV matmuls + transposes.
        # Row chains (var/sqrt/recip) are batched per 2 pgs (multi-lane).
        with tc.tile_pool(name="ph1ps", bufs=1, space="PSUM") as ph1:

            stats_ps_map = {}

            def emit_stats(pg):
                # s1 via a DVE add-tree + ONE ones-matmul; s2 via 8 ones-
                # matmuls over x^2 chunks (x^2 on the idle ScalarE).  Both
                # land in one [1,1024] PSUM row tile (s1 cols 0:512, s2
                # 512:1024).  GpSimd proved too slow for bulk reduction work.
                xa = x_tiles[pg]
                xsq = sqp.tile([128, 8, 512], BF16, tag="xsq", name=f"xsq{pg}")
                nc.scalar.activation(xsq[:], xa[:], AF.Square)
                y1 = trp.tile([128, 4, 512], BF16, tag="y1", name=f"y1_{pg}")
                nc.vector.tensor_add(y1[:], xa[:, 0:4, :], xa[:, 4:8, :])
                y2 = trp.tile([128, 2, 512], BF16, tag="y2", name=f"y2_{pg}")
                nc.vector.tensor_add(y2[:], y1[:, 0:2, :], y1[:, 2:4, :])
                yy = trp.tile([128, 512], BF16, tag="yy", name=f"yy{pg}")
                nc.vector.tensor_add(yy[:], y2[:, 0, :], y2[:, 1, :])
                sp = ph1.tile([1, 1024], F32, tag="stats", bufs=1, name=f"sp{pg}")
                nc.tensor.matmul(
                    sp[0:1, 0:512], lhsT=ones_col[:], rhs=yy[:],
                    start=True, stop=True, skip_group_check=True,
                )
                for hc in range(8):
                    nc.tensor.matmul(
                        sp[0:1, 512:1024], lhsT=ones_col[:], rhs=xsq[:, hc, :],
                        start=(hc == 0), stop=(hc == 7), skip_group_check=True,
                    )
                stats_ps_map[pg] = sp

            mu_rows = {}
            rstd_rows = {}
            rstdT_sb = {}

            def emit_rows(pg):
                # single-lane row chain from the PSUM stats row; sqrt and mu
                # on ScalarE, the rest on DVE; rstd broadcast on GpSimd
                sp = stats_ps_map[pg]
                s_sb = rowp.tile([1, 1024], F32, tag="ssb", name=f"ssb{pg}")
                nc.scalar.copy(s_sb[:], sp[:])
                s1 = s_sb[0:1, 0:512]
                s2 = s_sb[0:1, 512:1024]
                t_sq = rowp.tile([1, 512], F32, tag="tsq", name=f"tsq{pg}")
                nc.vector.tensor_mul(t_sq[:], s1, s1)
                u_t = rowp.tile([1, 512], F32, tag="u", name=f"u{pg}")
                nc.vector.scalar_tensor_tensor(
                    out=u_t[:], in0=t_sq[:], scalar=1.0 / HID,
                    in1=s2, op0=OP.mult, op1=OP.subtract,
                )  # u = s1^2/HID - s2  (= -HID*var)
                std_t = rowp.tile([1, 512], F32, tag="std", name=f"std{pg}")
                nc.scalar.activation(
                    std_t[:], u_t[:], AF.Sqrt, bias=eps_t[:], scale=-1.0 / HID
                )
                rstd_row = rowp.tile([1, 512], F32, tag="rstd", name=f"rstd{pg}")
                nc.vector.reciprocal_approx_fast(out=rstd_row[:], in_=std_t[:])
                rstd_b = bcp.tile([128, 512], F32, tag="rstdb", name=f"rstdb{pg}")
                nc.gpsimd.partition_broadcast(rstd_b[:], rstd_row[:])
                rstd_rows[pg] = (rstd_row, rstd_b)
                mu_t = rowp.tile([1, 512], BF16, tag="mu", name=f"mu{pg}")
                nc.scalar.mul(mu_t[:], s1, -1.0 / HID)
                mu_rows[pg] = mu_t
                # rstd columns for the V scaling: 4 tiny transposes into one
                # PSUM tile, evacuated with a single [128,4] copy
                tp = ph1.tile([128, 4], F32, tag="rTp", bufs=1, name=f"rTp{pg}")
                for c in range(4):
                    nc.tensor.matmul(
                        tp[:, c : c + 1], rstd_row[:, c * 128 : (c + 1) * 128],
                        ident32[0:1, 0:1], is_transpose=True,
                        skip_group_check=True,
                    )
                rT = bcp.tile([128, 4], F32, tag="rT", name=f"rT{pg}")
                nc.vector.tensor_copy(rT[:], tp[:])
                rstdT_sb[pg] = rT

            def emit_qkv(pg):
                xa = x_tiles[pg]
                rstd_b = rstd_rows[pg][1]
                ps = pg * 512
                vT_blk = epp.tile([128, 512], BF16, tag="vT", name=f"vT{pg}")
                # with bias, V must be rstd-scaled before the bias add, so it
                # takes the DVE stt path; otherwise V is evacuated raw on the
                # idle ScalarE and rstd is applied after the transpose.
                for (w_sb, wcol, target, scaled) in (
                    (wq_sb, 0, qT_sb[:, ps : ps + 512], True),
                    (wk_sb, 1, kT_sb[:, ps : ps + 512], True),
                    (wv_sb, 2, vT_blk[:], bool(with_bias)),
                ):
                    mm_ps = ph1.tile([128, 512], F32, tag="mm", bufs=3)
                    for hc in range(8):
                        nc.tensor.matmul(
                            mm_ps[:], lhsT=w_sb[:, hc, :], rhs=xa[:, hc, :],
                            start=(hc == 0), stop=False,
                        )
                    nc.tensor.matmul(
                        mm_ps[:],
                        lhsT=wsumsT_sb[:, wcol * DL : (wcol + 1) * DL],
                        rhs=mu_rows[pg][:], start=False, stop=True,
                    )
                    if scaled:
                        # target = raw_c * rstd (per-position column scale)
                        if with_bias:
                            t2 = epp.tile([128, 512], F32, tag="ep2")
                            nc.vector.scalar_tensor_tensor(
                                out=t2[:], in0=rstd_b[:], scalar=1.0,
                                in1=mm_ps[:], op0=OP.mult, op1=OP.mult,
                            )
                            nc.vector.tensor_scalar_add(
                                out=target, in0=t2[:],
                                scalar1=bqkv_sb[:, wcol : wcol + 1],
                            )
                        else:
                            nc.vector.scalar_tensor_tensor(
                                out=target, in0=rstd_b[:], scalar=1.0,
                                in1=mm_ps[:], op0=OP.mult, op1=OP.mult,
                            )
                    else:
                        # raw vT evacuation on the idle ScalarE; rstd is
                        # applied per-partition after the transpose
                        nc.scalar.copy(target, mm_ps[:])
                # transpose vT -> v (per 128-pos chunk) into padded v tiles,
                # scaling by rstd (now per-partition) on the way out
                for c4 in range(4):
                    j = pg * 4 + c4  # global 128-chunk == b*KC + kc
                    tp_ps = ph1.tile([128, 128], BF16, tag="vt", bufs=2)
                    nc.tensor.transpose(
                        tp_ps[:, :], vT_blk[:, c4 * 128 : (c4 + 1) * 128], ident[:]
                    )
                    if with_bias:
                        nc.vector.tensor_copy(vA_sb[:, j, 0:HD], tp_ps[:, 0:HD])
                        nc.vector.tensor_copy(
                            vB_sb[:, j, 0:HD], tp_ps[:, HD : 2 * HD]
                        )
                    else:
                        rcol = rstdT_sb[pg][:, c4 : c4 + 1]
                        nc.vector.tensor_scalar_mul(
                            out=vA_sb[:, j, 0:HD], in0=tp_ps[:, 0:HD], scalar1=rcol
                        )
                        nc.vector.tensor_scalar_mul(
                            out=vB_sb[:, j, 0:HD], in0=tp_ps[:, HD : 2 * HD],
                            scalar1=rcol,
                        )

            # software pipeline: stats/rows for pg+2/pg+1 emitted before
            # qkv(pg) so DVE/GpSimd/Scalar run ahead of the PE
            emit_stats(0)
            emit_rows(0)
            emit_stats(1)
            emit_rows(1)
            for pg in range(NPOSG):
                nxt = pg + 2
                if nxt < NPOSG:
                    x_tiles[nxt] = xpool.tile(
                        [128, 8, 512], BF16, tag="x", name=f"x{nxt}"
                    )
                    nc.sync.dma_start(
                        out=x_tiles[nxt][:],
                        in_=xT_r[:, :, nxt * 512 : nxt * 512 + 512],
                    )
                    emit_stats(nxt)
                    emit_rows(nxt)
                emit_qkv(pg)

        # ================= phase 2: attention + pipelined dense ==============
        with tc.tile_pool(name="scps", bufs=1, space="PSUM") as scps, \
             tc.tile_pool(name="ctps", bufs=1, space="PSUM") as ctps, \
             tc.tile_pool(name="dsps", bufs=1, space="PSUM") as dsps:

            def emit_dense_piece(qs, piece):
                # one (pos-chunk, half) slice of the dense projection; pieces
                # are woven one-per-unit into the NEXT query group so the
                # 8-matmul dense block never head-of-line-blocks the next
                # group's scores in the in-order PE queue
                c4, half = piece // 2, piece % 2
                pc = qs + c4 * 128
                ops_ = dsps.tile([128, 512], F32, tag="ds", bufs=2)
                nc.tensor.matmul(
                    ops_[:], lhsT=ctxT_sb[:, pc : pc + 128],
                    rhs=wd_sb[:, half * 512 : (half + 1) * 512],
                    start=True, stop=True,
                )
                osb = obp.tile([128, 512], F16, tag="ob")
                nc.vector.tensor_copy(osb[:], ops_[:])
                nc.sync.dma_start(
                    out=out[pc : pc + 128, half * 512 : (half + 1) * 512],
                    in_=osb[:],
                )

            pending_qs = None
            for b in range(B):
                for qg in range(QG):
                    qs = b * S + qg * 512
                    ctxA_ps = ctps.tile([128, 512], F32, tag="ctx", bufs=2)
                    ctxB_ps = ctps.tile([128, 512], F32, tag="ctx", bufs=2)
                    # per-head sub-units (scores -> exp -> pv) so the two sc
                    # slots recycle alternately and ScalarE never bubbles
                    for kc2 in range(KC // 2):
                        kc = 2 * kc2
                        ks = b * S + kc * 128
                        st = kc == 0
                        sp2 = kc + 1 == KC - 1
                        for h, (kh, vh, cps) in enumerate(
                            ((slice(0, 64), vA_sb, ctxA_ps),
                             (slice(64, 128), vB_sb, ctxB_ps))
                        ):
                            u = 2 * kc2 + h
                            if pending_qs is not None and 6 <= u < 14:
                                emit_dense_piece(pending_qs, u - 6)
                            elif pending_qs is None and u < 12:
                                # first query group has no woven dense: keep
                                # the PE stream dense with throwaway matmuls
                                # so it ramps to full clock instead of
                                # settling into a slow-PE/late-scores loop
                                dmy = dsps.tile(
                                    [128, 512], F32, tag="ds", bufs=2,
                                    name=f"dmy{u}",
                                )
                                nc.tensor.matmul(
                                    dmy[:], lhsT=ident[:],
                                    rhs=wd_sb[:, 0:512],
                                    start=True, stop=True,
                                )
                            psH = scps.tile(
                                [128, 1024], F32, tag="sc", bufs=2, name=f"ps{h}"
                            )
                            for j in range(2):
                                nc.tensor.matmul(
                                    psH[:, 512 * j : 512 * (j + 1)],
                                    lhsT=kT_sb[kh, ks + 128 * j : ks + 128 * (j + 1)],
                                    rhs=qT_sb[kh, qs : qs + 512],
                                    start=True, stop=True,
                                )
                            eH = etp.tile([128, 1024], BF16, tag="e", name=f"e{h}")
                            if fused_mask:
                                nc.scalar.activation(eH[:], psH[:], AF.Exp)
                            else:
                                for j in range(2):
                                    mcol = madd_sb[
                                        :, b * KC + kc + j : b * KC + kc + j + 1
                                    ]
                                    nc.scalar.activation(
                                        eH[:, 512 * j : 512 * (j + 1)],
                                        psH[:, 512 * j : 512 * (j + 1)],
                                        AF.Exp, bias=mcol, scale=1.0,
                                    )
                            for j in range(2):
                                nc.tensor.matmul(
                                    cps[:, :],
                                    lhsT=vh[:, b * KC + kc + j, :],
                                    rhs=eH[:, 512 * j : 512 * (j + 1)],
                                    start=(st and j == 0), stop=(sp2 and j == 1),
                                )

                    # evacuate ctx PSUM fast (frees the accum slots for the
                    # next query group), then normalize from the SBUF copies:
                    # head A ctx -> partitions 0:64, head B ctx -> 64:128 so the
                    # normalize muls have partition-aligned SBUF operands
                    cAB = dnp.tile([128, 512], F32, tag="cs", bufs=2)
                    nc.vector.tensor_copy(cAB[0:HD, :], ctxA_ps[0:HD, :])
                    nc.vector.tensor_copy(cAB[HD : 2 * HD, :], ctxB_ps[0:HD, :])
                    dn_row = dnp.tile([1, 1024], F32, tag="dn_row", bufs=2)
                    nc.vector.tensor_copy(dn_row[:, 0:512], ctxA_ps[HD : HD + 1, :])
                    nc.vector.tensor_copy(dn_row[:, 512:1024], ctxB_ps[HD : HD + 1, :])
                    rdn_row = dnp.tile([1, 1024], F32, tag="rdn_row", bufs=1)
                    nc.vector.reciprocal_approx_fast(out=rdn_row[:], in_=dn_row[:])
                    rdn_b = dnp.tile([128, 1024], F32, tag="rdn_b", bufs=1)
                    nc.gpsimd.partition_broadcast(rdn_b[:], rdn_row[:])
                    nc.vector.tensor_mul(
                        ctxT_sb[0:HD, qs : qs + 512],
                        cAB[0:HD, :], rdn_b[0:HD, 0:512],
                    )
                    nc.vector.tensor_mul(
                        ctxT_sb[HD : 2 * HD, qs : qs + 512],
                        cAB[HD : 2 * HD, :], rdn_b[HD : 2 * HD, 512:1024],
                    )
                    pending_qs = qs
            # dense for the final query group has no successor to hide in
            for piece in range(8):
                emit_dense_piece(pending_qs, piece)
    nc.compile()
    return nc


def _get_nc(with_bias, fused_mask):
    key = (bool(with_bias), bool(fused_mask))
    if key not in _BUILT:
        _BUILT[key] = _build(*key)
    return _BUILT[key]


def kernel(
    hidden_states,
    attention_mask,
    Wq, bq, Wk, bk, Wv, bv, Wd, bd,
    ln_gamma, ln_beta,
):
    from concourse.bass_utils import run_bass_kernel_spmd

    hidden_states = np.asarray(hidden_states, dtype=np.float32)
    attention_mask = np.asarray(attention_mask, dtype=np.float32)
    Wq, bq = np.asarray(Wq, np.float32), np.asarray(bq, np.float32)
    Wk, bk = np.asarray(Wk, np.float32), np.asarray(bk, np.float32)
    Wv, bv = np.asarray(Wv, np.float32), np.asarray(bv, np.float32)
    Wd, bd = np.asarray(Wd, np.float32), np.asarray(bd, np.float32)
    gamma = np.asarray(ln_gamma, np.float32)
    beta = np.asarray(ln_beta, np.float32)

    x2d = hidden_states.reshape(PB, HID)
    xT = np.ascontiguousarray(x2d.T).astype(ml_dtypes.bfloat16)

    ma = (-1000.0 * (1.0 - attention_mask)).astype(np.float32)  # [B, S]
    madd = np.ascontiguousarray(
        ma.reshape(B, KC, 128).transpose(2, 0, 1).reshape(128, B * KC)
    )
    fused_mask = not np.any(ma != 0)

    in_maps = []
    biases_eff = []
    for p in range(NCORES):
        sl = slice(DL * p, DL * (p + 1))
        wq_e = Wq[sl, :] * gamma[None, :] * np.float32(SCALE)
        wk_e = Wk[sl, :] * gamma[None, :]
        wv_e = Wv[sl, :] * gamma[None, :]
        wq_b = np.ascontiguousarray(wq_e.T).astype(ml_dtypes.bfloat16)
        wk_b = np.ascontiguousarray(wk_e.T).astype(ml_dtypes.bfloat16)
        wv_b = np.ascontiguousarray(wv_e.T).astype(ml_dtypes.bfloat16)
        # raw column sums of the bf16 weights actually used on device,
        # as three [1, DL] rows for the rank-1 mean-correction matmul
        wsumsT = np.concatenate(
            [
                wq_b.astype(np.float32).sum(axis=0),
                wk_b.astype(np.float32).sum(axis=0),
                wv_b.astype(np.float32).sum(axis=0),
            ]
        ).reshape(1, 3 * DL).astype(ml_dtypes.bfloat16)
        b_eff = np.stack(
            [
                (Wq[sl, :] @ beta + bq[sl]) * np.float32(SCALE),
                Wk[sl, :] @ beta + bk[sl],
                Wv[sl, :] @ beta + bv[sl],
            ],
            axis=1,
        ).astype(np.float32)
        biases_eff.append(b_eff)
        wd_s = np.ascontiguousarray(Wd[:, sl].T).astype(ml_dtypes.bfloat16)
        in_maps.append(
            {
                "xT": xT,
                "wq": wq_b,
                "wk": wk_b,
                "wv": wv_b,
                "wd": wd_s,
                "wsumsT": wsumsT,
            }
        )

    with_bias = any(np.any(b != 0) for b in biases_eff)
    if with_bias:
        for p in range(NCORES):
            in_maps[p]["bqkv"] = biases_eff[p]
    if not fused_mask:
        for p in range(NCORES):
            in_maps[p]["madd"] = madd

    nc = _get_nc(with_bias, fused_mask)
    last_launch["nc"] = nc
    last_launch["in_maps"] = in_maps
    res = run_bass_kernel_spmd(nc, in_maps, core_ids=list(range(NCORES)))
    acc = res.results[0]["out"].astype(np.float32).copy()
    for p in range(1, NCORES):
        acc += res.results[p]["out"]
    acc += bd[None, :]
    return acc.reshape(B, S, HID)
